# revision 3
# baseline (speedup 1.0000x reference)
"""ALiBi bias kernel for Trainium2, SPMD across 8 NeuronCores.

Output: bias[h, i, j] = -slopes[h] * (j - i) if j > i else 0, for
h in [0, 16), i, j in [0, 4096).  ~1 GiB of f32, head-parallel across
8 cores (2 heads per core).

Strategy: within one head, output row i is a shifted copy of the ramp
v[d] = -slope * relu(d).  A "skewed" table
    tbl[p, x] = -slope * relu(x - p),   p in [0,128), x in [0,4096)
has the property that for the 128-row output tile starting at row
i0 = 128*t, element bias[i0 + p, i0 + x] equals tbl[p, x] exactly.
So each output tile is a plain SBUF->DRAM DMA of a prefix of the
table, running at HBM write bandwidth.

run_bass_kernel_spmd pre-zeroes ExternalOutput buffers (kernels that
don't write every element rely on that), so we only write the columns
j >= 128*t of each row block; everything left of that is in the
strictly-lower causal triangle and identically zero.  This halves the
written bytes (~69 MB/core instead of 128 MiB/core).

The tables are generated ON DEVICE by the otherwise-idle compute
engines (gpsimd iota makes x - p, one fused vector tensor_scalar per
head computes min(-slope*(x-p), 0) = -slope*relu(x-p)), chunked so the
store stream starts within ~2.5 us of the preamble.  Only a 1 KB
slopes vector is DMA'd in.  Stores are issued small-tile-first on both
HWDGE rings (SP via nc.sync, Activation via nc.scalar) so the
descriptor-bound small tiles overlap table generation and the run
drains on large line-rate DMAs.
"""

import sys

if "/opt/trn_rl_repo" not in sys.path:
    sys.path.insert(0, "/opt/trn_rl_repo")

import numpy as np

import concourse.bass as bass
import concourse.mybir as mybir
from concourse.bass_utils import run_bass_kernel_spmd

N_CORES = 8
N_HEADS = 16
HPC = N_HEADS // N_CORES  # heads per core
S = 4096  # seq_len
P = 128  # SBUF partitions / rows per tile
NT = S // P  # tiles per head
CHUNK = 1024  # table-generation chunk (columns)
NCHUNK = S // CHUNK

# ring assignment: groups of 4 tiles {4k,4k+1,4k+2,4k+3} split as
# A={4k, 4k+3}, B={4k+1, 4k+2} -> equal bytes per ring in every group
RING_A = sorted((t for t in range(NT) if t % 4 in (0, 3)), reverse=True)
RING_B = sorted((t for t in range(NT) if t % 4 in (1, 2)), reverse=True)

_cache: dict = {}


def _chunks_needed(t: int) -> int:
    w = S - P * t
    return -(-w // CHUNK)  # ceil


def _build() -> bass.Bass:
    f32 = mybir.dt.float32
    nc = bass.Bass()
    negslope_ext = nc.declare_dram_parameter("negslope", [P, HPC], f32, isOutput=False)
    out_ext = nc.declare_dram_parameter("out", [HPC, S, S], f32, isOutput=True)

    with (
        nc.sbuf_tensor([P, HPC * S], f32) as tbl,
        nc.sbuf_tensor([P, S], f32) as base,
        nc.sbuf_tensor([P, HPC], f32) as negslope,
        nc.semaphore("slopes_sem") as slopes_sem,
        nc.semaphore("iota_sem") as iota_sem,
        nc.semaphore("gen_sem") as gen_sem,
        nc.semaphore("storeA") as storeA,
        nc.semaphore("storeB") as storeB,
        nc.Block() as block,
    ):

        @block.gpsimd
        def _(gpsimd):
            for c in range(NCHUNK):
                gpsimd.iota(
                    base[:, c * CHUNK : (c + 1) * CHUNK],
                    pattern=[[1, CHUNK]],
                    base=c * CHUNK,
                    channel_multiplier=-1,
                    allow_small_or_imprecise_dtypes=True,
                ).then_inc(iota_sem, 1)

        @block.vector
        def _(vector):
            vector.wait_ge(slopes_sem, 16)
            for c in range(NCHUNK):
                vector.wait_ge(iota_sem, c + 1)
                for h in range(HPC):
                    vector.tensor_scalar(
                        tbl[:, h * S + c * CHUNK : h * S + (c + 1) * CHUNK],
                        base[:, c * CHUNK : (c + 1) * CHUNK],
                        scalar1=negslope[:, h : h + 1],
                        scalar2=0.0,
                        op0=mybir.AluOpType.mult,
                        op1=mybir.AluOpType.min,
                    ).then_inc(gen_sem, 1)

        def store_tile(eng, t):
            # one DMA covering both heads' rows [128t, 128t+128) x cols [128t, S)
            w = S - P * t
            src = tbl[:, :].rearrange("p (h x) -> p h x", h=HPC)[:, :, :w]
            dst = out_ext[:, P * t : P * (t + 1), P * t : S].transpose([1, 0, 2])
            return eng.dma_start(out=dst, in_=src)

        def ring(eng, tiles, store_sem):
            gen_have = 0
            n = 0
            for t in tiles:
                need = HPC * _chunks_needed(t)
                if need > gen_have:
                    eng.wait_ge(gen_sem, need)
                    gen_have = need
                store_tile(eng, t).then_inc(store_sem, 16)
                n += 1
            eng.wait_ge(store_sem, 16 * n)

        @block.sync
        def _(sync):
            sync.dma_start(out=negslope[:, :], in_=negslope_ext[:, :]).then_inc(
                slopes_sem, 16
            )
            ring(sync, RING_A, storeA)

        @block.scalar
        def _(scalar):
            ring(scalar, RING_B, storeB)

    return nc


def _get_nc() -> bass.Bass:
    if "nc" not in _cache:
        _cache["nc"] = _build()
    return _cache["nc"]


def make_in_maps(slopes: np.ndarray):
    """Per-core input maps: a [128, HPC] buffer holding -slope per head,
    replicated across all 128 partitions (per-partition scalar operand)."""
    slopes = np.asarray(slopes, dtype=np.float32)
    maps = []
    for c in range(N_CORES):
        neg = -slopes[HPC * c : HPC * (c + 1)]  # [HPC]
        maps.append({"negslope": np.ascontiguousarray(np.tile(neg, (P, 1)))})
    return maps


def kernel(slopes: np.ndarray, seq_len) -> np.ndarray:
    assert int(seq_len) == S, f"kernel hardcoded for seq_len={S}, got {seq_len}"
    slopes = np.asarray(slopes, dtype=np.float32)
    assert slopes.shape == (N_HEADS,)

    nc = _get_nc()
    res = run_bass_kernel_spmd(nc, make_in_maps(slopes), list(range(N_CORES)))
    out = np.concatenate([res.results[c]["out"] for c in range(N_CORES)], axis=0)
    return out
